# revision 15
# baseline (speedup 1.0000x reference)
"""Trainium2 Bass kernel for ChaoticEmbedding (Lorenz RK4 trajectory).

Reference computation:
  - three tiny MLPs map features[B,4] -> init state (3), coupling (3),
    adapted lorenz params (3)
  - 49 RK4 steps of the coupled Lorenz ODE, trajectory of all 50 states out.

Strategy: pure data-parallel across 8 NeuronCores (batch dim).  Per core
32768 samples laid out as [128 partitions x 256 free].  Everything is
elementwise per sample, so the main loop is VectorE(DVE)-bound.  Key
tricks:
  - variable change z' = z - c2/b, r' = r - c2/b removes the c2 coupling
    term from the RHS entirely (it reappears only in the trajectory
    write, one strided add per step).
  - per-sample quantities live in fixed column "blocks" of one big SBUF
    tensor; operands of each DVE instruction are constant-stride groups
    of blocks, so one Lorenz RHS evaluation is only 3 wide DVE
    instructions (minimum element-work given the 2-read-port ISA):
      [t1|q|cy]  = [y|r'|c1]   - [x|z|y]       (768 wide, sub)
      [w|p|v|u]  = [nb|x|y|s] * [z|q|x|t1]     (1024 wide, mul)
      [dx|dz|dy] = [u|v|p]    + [c0|w|cy]      (768 wide, add)
  - the evals run on an fp16 mirror frame (all-16-bit tensor_tensor ->
    DVE 2x_1P perf mode, half the cycles); state, RK4 accumulation and
    the trajectory stay fp32.  Measured output error vs the fp32 jax
    reference: 3.5e-3 scale-relative (fp32-eval fallback via
    USE_BF16_EVAL=False: 1.6e-5).
  - RK4 stage inputs / accumulator are fused scalar_tensor_tensor ops:
    Y = (K*h)+S (written to the fp16 mirror directly), A = (K*w)+A.
  - trajectory accumulates in a double-buffered SBUF chunk buffer,
    DMA-flushed to DRAM in 8-step chunks overlapped with compute; x/y
    trajectory copies and the per-step state->fp16 mirror cast run on
    the otherwise idle ScalarE.

Per-step DVE work: 4 fp16 evals x (535+663+535)c + 7 STT x 919c +
zfix 407c = ~13.8k cycles at 0.96 GHz = ~14.4us; x49 steps + MLP
prologue; ~1.0 ms/core measured end-to-end on hardware.
"""

import numpy as np

import concourse.bacc as bacc
import concourse.mybir as mybir
import concourse.tile as tile

# Problem constants (hardcoded per the harness contract).
B = 262144
D = 4
T = 50
HSTEP = 0.01
SIGMA, RHO, BETA = 10.0, 28.0, 8.0 / 3.0
N_CORES = 8
P = 128

FP = mybir.dt.float32
BF = mybir.dt.float16
ALU = mybir.AluOpType
ACTF = mybir.ActivationFunctionType

# Run the 4 Lorenz RHS evals per step on an fp16 mirror of the stage state:
# all-16-bit tensor_tensor ops hit the DVE 2x_1P perf mode (half the
# cycles).  State, RK4 accumulation and the trajectory stay fp32 (the
# stage-combine STTs compute in fp32 internally and round once to fp16 on
# output), so fp16 noise only enters through the k_i values.  fp16 (10
# mantissa bits) instead of bf16: 4x less rounding noise, and the dynamic
# range here (|values| < ~2000) is far from fp16 limits.
USE_BF16_EVAL = True

# Trajectory downloaded as fp16 (half the device->host bytes; the final
# fp32 cast happens on the host, per-shard, overlapped with the other
# downloads).  fp16 rounding of the output adds only ~5e-4 relative error.
# Features stay fp32: rounding the *inputs* to fp16 gets amplified by the
# chaotic dynamics (measured 3.2e-3 -> 1.1e-2 total), not worth 2MB.
USE_FP16_OUT = True
OUT_DT = BF if USE_FP16_OUT else FP
FEAT_DT = FP
FEAT_NP = np.float32

# ---- column block map (units of F columns) -------------------------------
# fp16 eval frame layout (22 blocks), satisfying every constant-stride
# operand-group constraint:
#   op2  in0 (y,r',c1) stride -7 ; in1 (x,z,y) stride 2 ; out (t1,q,cy) str 2
#   op34 in0 (nb,x,y,s) stride 4 ; in1 (z,q,x,t1) stride -1 ;
#        out (w,p,v,u) stride -1
#   op5  in0 (u,v,p) stride 1 ; in1 (c0,w,cy) stride 8 ; out K contiguous
OFF = {"c0": 0, "c1": 3, "u": 5, "v": 6, "p": 7, "w": 8, "nb": 9,
       "rp": 10, "t1": 12, "x": 13, "q": 14, "z": 15, "cy": 16,
       "y": 17, "s": 21}
FRSZ = 22
# fp32 side: state frames are contiguous (x,z,y at +0,+1,+2); statics sit
# at FR0+3.. in STATICS order (only used in the prologue / as cast source).
FR0, FR1, FR2 = 0, FRSZ, 2 * FRSZ          # frame bases
K_ = 3 * FRSZ                               # 66,67,68 = (dx, dz, dy)
CZB = K_ + 3                                # 69
FEAT = 70                                   # 70..73 raw interleaved features
HB = 74                                     # 74..89 MLP hidden scratch
NACC = 4
ACCB = [90, 91, 92, 93]                     # rotating MLP accumulators
SIG = [94, 95, 96]                          # param-MLP sigmoid outputs
CC = [97, 98, 99]                           # coupling-MLP outputs c0,c1,c2
TMP = 74                                    # post-MLP scratch (h dead)
NBLK = 100

# statics replicated into the eval frame (frame-relative offsets)
STATICS = ["c0", "c1", "nb", "rp", "s"]

# weight table offsets inside the broadcast WT tensor
_off = {}
_cur = 0
for _name, _n in [("W1", 64), ("b1", 16), ("W2", 48), ("b2", 3),
                  ("Wc1", 32), ("bc1", 8), ("Wc2", 24), ("bc2", 3),
                  ("Wp1", 32), ("bp1", 8), ("Wp2", 24), ("bp2", 3)]:
    _off[_name] = _cur
    _cur += _n
WT_COLS = 320


def _mk(base_ap, offset, dims):
    """Custom AP: keep partition dim of base_ap, set free dims/offset."""
    a = base_ap.copy()
    v = a.ap
    part = tuple(v.to_list()[0])
    v.clear()
    v.append(part)
    for step, count in dims:
        v.append((int(step), int(count)))
    a.offset = int(offset)
    return a


def build_kernel(tc, out_ap, ins, n_samples, n_steps):
    """Emit the per-core kernel.  ins: dict name->AP of DRAM inputs."""
    nc = tc.nc
    F = n_samples // P
    assert n_samples % P == 0

    big = nc.alloc_sbuf_tensor("big", [P, NBLK * F], FP).ap()
    # trajectory chunk buffer: double-buffered, CH steps per chunk,
    # per-chunk layout f = i*(3*CH) + tt*3 + v  (sample-major inside chunk)
    CH = 8
    traj = nc.alloc_sbuf_tensor("traj", [P, 2 * 3 * CH * F], OUT_DT).ap()
    wt = nc.alloc_sbuf_tensor("wt", [P, WT_COLS], FP).ap()
    if USE_BF16_EVAL:
        # bf16 eval mirror: one frame (same OFF layout) + K16 at +FRSZ
        K16 = FRSZ
        big16 = nc.alloc_sbuf_tensor("big16", [P, (FRSZ + 3) * F], BF).ap()

    def blk(i, n=1):
        return big[:, i * F:(i + n) * F]

    def _grp_on(tens, blocks, width=None):
        """Constant-stride group AP over blocks (offsets in F units)."""
        w = F if width is None else width
        if len(blocks) == 1:
            return tens[:, blocks[0] * F: blocks[0] * F + w]
        step = blocks[1] - blocks[0]
        for a, b in zip(blocks, blocks[1:]):
            assert b - a == step, blocks
        return _mk(tens, blocks[0] * F, [(step * F, len(blocks)), (1, w)])

    def grp(blocks, width=None):
        return _grp_on(big, blocks, width)

    def fgrp(base, names):
        return grp([base + OFF[n] for n in names])

    if USE_BF16_EVAL:
        # f32 state is contiguous (x,z,y at +0..+2); statics at FR0+3..
        SX, SZ, SY = 0, 1, 2
        SOFF = {"c0": 3, "c1": 4, "nb": 5, "rp": 6, "s": 7}

        def egrp(blocks, width=None):
            return _grp_on(big16, blocks, width)

        def efgrp(names):
            return egrp([OFF[n] for n in names])

        def sgrp(base):
            return grp([base], width=3 * F)
    else:
        SX, SZ, SY = OFF["x"], OFF["z"], OFF["y"]
        SOFF = OFF

        def sgrp(base):
            return fgrp(base, ["x", "z", "y"])

    # ---------------- prologue: load inputs ------------------------------
    # features [n_samples, 4] -> [P, 4F] (contiguous per partition)
    nc.sync.dma_start(out=blk(FEAT, 4),
                      in_=ins["features"].rearrange("(p i) d -> p (i d)", p=P))
    # broadcast all weights/biases to every partition
    for name in _off:
        src = ins[name].flatten().unsqueeze(0)
        n = src.shape[1]
        nc.sync.dma_start(out=wt[:, _off[name]:_off[name] + n],
                          in_=src.broadcast_to((P, n)))

    f = [big[:, FEAT * F + k: (FEAT + 4) * F: 4] for k in range(4)]

    acc_rot = [0]

    def mlp(wkey, bkey, w2key, b2key, nhid, act1, act2, outblks):
        """Tiny MLP on DVE/ACT: out_i = act2(sum_j act1(f@W1)_j W2[j,i] + b2).

        The accumulator rotates over NACC blocks so the ScalarE init of
        unit i+1 pipelines with the DVE STT chain of unit i."""
        def unit(inputs, woff, wstride, bo, actf, outblk):
            a = ACCB[acc_rot[0] % NACC]
            acc_rot[0] += 1
            nc.scalar.mul(blk(a), inputs[0], wt[:, woff:woff + 1])
            for k in range(1, len(inputs)):
                wo = woff + k * wstride
                nc.vector.scalar_tensor_tensor(
                    out=blk(a), in0=inputs[k],
                    scalar=wt[:, wo:wo + 1],
                    in1=blk(a), op0=ALU.mult, op1=ALU.add)
            nc.scalar.activation(blk(outblk), blk(a), actf,
                                 bias=wt[:, bo:bo + 1])

        hblks = list(range(HB, HB + nhid))
        for j in range(nhid):
            unit(f, _off[wkey] + j, nhid, _off[bkey] + j, act1, hblks[j])
        hin = [blk(h) for h in hblks]
        for i in range(3):
            unit(hin, _off[w2key] + i, 3, _off[b2key] + i, act2, outblks[i])

    # param MLP -> sigmoid scales; coupling MLP -> c0,c1,c2
    mlp("Wp1", "bp1", "Wp2", "bp2", 8, ACTF.Relu, ACTF.Sigmoid, SIG)
    mlp("Wc1", "bc1", "Wc2", "bc2", 8, ACTF.Tanh, ACTF.Tanh, CC)
    # init-state MLP -> raw tanh in (x, y, z) order -> frame0 state slots
    XB, ZB, YB = FR0 + SX, FR0 + SZ, FR0 + SY
    mlp("W1", "b1", "W2", "b2", 16, ACTF.Tanh, ACTF.Tanh, [XB, YB, ZB])
    for pos in (XB, YB, ZB):
        nc.vector.tensor_scalar(blk(pos), blk(pos), 2.0, None, ALU.mult)

    ACC0, ACC1 = ACCB[0], ACCB[1]
    # derived params into frame0:
    # s = (sig0 + 0.5)*SIGMA ; nb = (sig2 + 0.5)*(-BETA)
    nc.vector.tensor_scalar(blk(FR0 + SOFF["s"]), blk(SIG[0]), 0.5, SIGMA,
                            ALU.add, ALU.mult)
    nc.vector.tensor_scalar(blk(FR0 + SOFF["nb"]), blk(SIG[2]), 0.5, -BETA,
                            ALU.add, ALU.mult)
    # czb = c2 / b = -(c2 * (1/nb))
    nc.vector.reciprocal(blk(ACC0), blk(FR0 + SOFF["nb"]))
    nc.vector.tensor_tensor(out=blk(ACC1), in0=blk(CC[2]), in1=blk(ACC0),
                            op=ALU.mult)
    nc.vector.tensor_scalar(blk(CZB), blk(ACC1), -1.0, None, ALU.mult)
    # r' = (sig1 + 0.5)*RHO - czb
    nc.vector.tensor_scalar(blk(ACC0), blk(SIG[1]), 0.5, RHO,
                            ALU.add, ALU.mult)
    nc.vector.tensor_tensor(out=blk(FR0 + SOFF["rp"]), in0=blk(ACC0),
                            in1=blk(CZB), op=ALU.subtract)
    # c0, c1 -> frame0
    nc.scalar.copy(blk(FR0 + SOFF["c0"]), blk(CC[0]))
    nc.scalar.copy(blk(FR0 + SOFF["c1"]), blk(CC[1]))

    # trajectory t=0 (before the z shift)
    def traj_out(t, v):
        base = ((t // CH) % 2) * 3 * CH * F
        start = base + (t % CH) * 3 + v
        return traj[:, start: base + 3 * CH * F: 3 * CH]

    nc.scalar.copy(traj_out(0, 0), blk(XB))
    nc.scalar.copy(traj_out(0, 1), blk(YB))
    nc.scalar.copy(traj_out(0, 2), blk(ZB))
    # z' = z - czb
    nc.vector.tensor_tensor(out=blk(ZB), in0=blk(ZB), in1=blk(CZB),
                            op=ALU.subtract)
    if USE_BF16_EVAL:
        # cast static params into the fp16 eval frame (once)
        for name in STATICS:
            nc.scalar.copy(egrp([OFF[name]]), blk(FR0 + SOFF[name]))
    else:
        # distribute statics to frames 1, 2
        for fr in (FR1, FR2):
            for name in STATICS:
                nc.scalar.copy(blk(fr + OFF[name]), blk(FR0 + OFF[name]))

    # ---------------- main loop ------------------------------------------
    if USE_BF16_EVAL:
        def lorenz_eval(src):
            # all-bf16 tensor_tensor ops -> DVE 2x_1P mode (half cycles)
            nc.vector.tensor_tensor(out=efgrp(["t1", "q", "cy"]),
                                    in0=efgrp(["y", "rp", "c1"]),
                                    in1=efgrp(["x", "z", "y"]),
                                    op=ALU.subtract)
            nc.vector.tensor_tensor(out=efgrp(["w", "p", "v", "u"]),
                                    in0=efgrp(["nb", "x", "y", "s"]),
                                    in1=efgrp(["z", "q", "x", "t1"]),
                                    op=ALU.mult)
            nc.vector.tensor_tensor(out=egrp([K16, K16 + 1, K16 + 2]),
                                    in0=efgrp(["u", "v", "p"]),
                                    in1=efgrp(["c0", "w", "cy"]),
                                    op=ALU.add)

        def kin():
            return egrp([K16], width=3 * F)

        def axpy_y(src_f32, k_scale):
            # EV16.(x,z,y) = (K16 * h) + S  (fp32 internal, one rounding)
            nc.vector.scalar_tensor_tensor(
                out=efgrp(["x", "z", "y"]), in0=kin(),
                scalar=float(k_scale),
                in1=sgrp(src_f32),
                op0=ALU.mult, op1=ALU.add)
    else:
        def lorenz_eval(src):
            # [t1|q|cy] = [y|r'|c1] - [x|z|y]
            nc.vector.tensor_tensor(out=fgrp(src, ["t1", "q", "cy"]),
                                    in0=fgrp(src, ["y", "rp", "c1"]),
                                    in1=fgrp(src, ["x", "z", "y"]),
                                    op=ALU.subtract)
            # [w|p|v|u] = [nb|x|y|s] * [z|q|x|t1]
            nc.vector.tensor_tensor(out=fgrp(src, ["w", "p", "v", "u"]),
                                    in0=fgrp(src, ["nb", "x", "y", "s"]),
                                    in1=fgrp(src, ["z", "q", "x", "t1"]),
                                    op=ALU.mult)
            # K(dx,dz,dy) = [u|v|p] + [c0|w|cy]
            nc.vector.tensor_tensor(out=grp([K_, K_ + 1, K_ + 2]),
                                    in0=fgrp(src, ["u", "v", "p"]),
                                    in1=fgrp(src, ["c0", "w", "cy"]),
                                    op=ALU.add)

        def kin():
            return grp([K_], width=3 * F)

        def axpy_y(src_f32, k_scale):
            nc.vector.scalar_tensor_tensor(
                out=fgrp(FR2, ["x", "z", "y"]), in0=kin(),
                scalar=float(k_scale),
                in1=fgrp(src_f32, ["x", "z", "y"]),
                op0=ALU.mult, op1=ALU.add)

    def axpy(dst, k_scale, addend):
        # dst.(x,z,y) = (K * k_scale) + addend.(x,z,y)   (fp32)
        nc.vector.scalar_tensor_tensor(
            out=sgrp(dst),
            in0=kin(),
            scalar=float(k_scale),
            in1=sgrp(addend),
            op0=ALU.mult, op1=ALU.add)

    out3 = out_ap.rearrange("(p i) c -> p i c", p=P)
    chunk_start = 0

    def flush(t_end):
        nonlocal chunk_start
        tc_n = t_end - chunk_start
        if tc_n <= 0:
            return
        base = ((chunk_start // CH) % 2) * 3 * CH * F
        sb = _mk(traj, base, [(3 * CH, F), (1, 3 * tc_n)])
        nc.sync.dma_start(
            out=out3[:, :, chunk_start * 3:t_end * 3], in_=sb)
        chunk_start = t_end

    def mirror(src):
        # refresh the bf16 eval mirror of the state (ScalarE, overlapped
        # with the DVE zfix so eval1 of the next step barely stalls)
        nc.scalar.copy(efgrp(["x", "z", "y"]), sgrp(src))

    EV = 0 if USE_BF16_EVAL else FR2
    sb_, ab_ = FR0, FR1
    if USE_BF16_EVAL:
        mirror(sb_)
    for t in range(1, n_steps):
        lorenz_eval(EV if USE_BF16_EVAL else sb_)       # k1
        axpy(ab_, HSTEP / 6.0, sb_)           # A  = S + h/6 k1
        axpy_y(sb_, HSTEP / 2.0)              # Y2 = S + h/2 k1
        lorenz_eval(EV)                       # k2
        axpy(ab_, HSTEP / 3.0, ab_)
        axpy_y(sb_, HSTEP / 2.0)              # Y3 = S + h/2 k2
        lorenz_eval(EV)                       # k3
        axpy(ab_, HSTEP / 3.0, ab_)
        axpy_y(sb_, HSTEP)                    # Y4 = S + h k3
        lorenz_eval(EV)                       # k4
        axpy(ab_, HSTEP / 6.0, ab_)           # A = S_new
        if USE_BF16_EVAL and t + 1 < n_steps:
            mirror(ab_)                       # bf16 mirror for step t+1
        # trajectory writes for step t
        nc.vector.tensor_tensor(out=traj_out(t, 2),
                                in0=blk(ab_ + SZ),
                                in1=blk(CZB), op=ALU.add)
        nc.scalar.copy(traj_out(t, 0), blk(ab_ + SX))
        nc.scalar.copy(traj_out(t, 1), blk(ab_ + SY))
        sb_, ab_ = ab_, sb_
        if (t + 1) % CH == 0:
            flush(t + 1)
    flush(n_steps)


_CACHE = {}


def _get_built(n_samples, n_steps):
    key = (n_samples, n_steps)
    if key in _CACHE:
        return _CACHE[key]
    nc = bacc.Bacc("TRN2", target_bir_lowering=False, debug=False,
                   enable_asserts=False)
    ins = {
        "features": nc.dram_tensor("features", [n_samples, D], FEAT_DT,
                                   kind="ExternalInput").ap(),
    }
    for name, shape in [("W1", [4, 16]), ("b1", [16]), ("W2", [16, 3]),
                        ("b2", [3]), ("Wc1", [4, 8]), ("bc1", [8]),
                        ("Wc2", [8, 3]), ("bc2", [3]), ("Wp1", [4, 8]),
                        ("bp1", [8]), ("Wp2", [8, 3]), ("bp2", [3])]:
        ins[name] = nc.dram_tensor(name, shape, FP,
                                   kind="ExternalInput").ap()
    out = nc.dram_tensor("traj_out", [n_samples, n_steps * 3], OUT_DT,
                         kind="ExternalOutput").ap()
    with tile.TileContext(nc) as tc:
        build_kernel(tc, out, ins, n_samples, n_steps)
    nc.compile()
    _CACHE[key] = nc
    return nc


# ---------------------------------------------------------------------------
# Dispatch: a hand-rolled, cached version of bass2jax.run_bass_via_pjrt.
# run_bass_kernel_spmd under axon rebuilds jax.jit objects (full retrace)
# every call, uploads full-output-sized host zero buffers per call for
# output donation, and splits/concatenates the output on the host (another
# full-size memcpy).  Here the jitted shard_map is built once and cached;
# the kernel writes every element of traj_out, so the outputs are plain
# (uninitialized) custom-call results with no donated operands at all (the
# scheme the bass_jit decorator path uses); and the output shards are
# fetched+cast per-device in threads straight into the preallocated fp32
# result (no concat, casts overlap downloads).
# ---------------------------------------------------------------------------
_DISPATCH = {}
_POOL = None


def _get_pool():
    global _POOL
    if _POOL is None:
        from concurrent.futures import ThreadPoolExecutor
        _POOL = ThreadPoolExecutor(N_CORES)
    return _POOL


def _get_dispatch(n_samples, n_steps):
    key = (n_samples, n_steps)
    if key in _DISPATCH:
        return _DISPATCH[key]
    import jax
    from jax.experimental.shard_map import shard_map
    from jax.sharding import Mesh, PartitionSpec
    from concourse import bass2jax

    bass2jax.install_neuronx_cc_hook()
    nc = _get_built(n_samples, n_steps)

    partition_name = (nc.partition_id_tensor.name
                      if nc.partition_id_tensor else None)
    in_names, out_names, out_avals = [], [], []
    for alloc in nc.m.functions[0].allocations:
        if not isinstance(alloc, mybir.MemoryLocationSet):
            continue
        name = alloc.memorylocations[0].name
        if alloc.kind == "ExternalInput":
            if name != partition_name:
                in_names.append(name)
        elif alloc.kind == "ExternalOutput":
            out_names.append(name)
            out_avals.append(jax.core.ShapedArray(
                tuple(alloc.tensor_shape), mybir.dt.np(alloc.dtype)))
    n_params = len(in_names)
    all_in_names = list(in_names)
    if partition_name is not None:
        all_in_names.append(partition_name)

    def _body(*args):
        operands = list(args)
        if partition_name is not None:
            operands.append(bass2jax.partition_id_tensor())
        outs = bass2jax._bass_exec_p.bind(
            *operands,
            out_avals=tuple(out_avals),
            in_names=tuple(all_in_names),
            out_names=tuple(out_names),
            lowering_input_output_aliases=(),
            sim_require_finite=True,
            sim_require_nnan=True,
            nc=nc,
        )
        return tuple(outs)

    devices = jax.devices()[:N_CORES]
    assert len(devices) == N_CORES
    mesh = Mesh(np.asarray(devices), ("core",))
    in_specs = (PartitionSpec("core"),) * n_params
    out_specs = (PartitionSpec("core"),) * len(out_names)
    sharded = jax.jit(
        shard_map(_body, mesh=mesh, in_specs=in_specs,
                  out_specs=out_specs, check_rep=False),
        keep_unused=True)
    entry = (sharded, in_names)
    _DISPATCH[key] = entry
    return entry


def kernel(features, W1, b1, W2, b2, Wc1, bc1, Wc2, bc2, Wp1, bp1,
           Wp2, bp2):
    n_full = features.shape[0]
    n_samples = n_full // N_CORES
    sharded, in_names = _get_dispatch(n_samples, T)
    vals = {"W1": W1, "b1": b1, "W2": W2, "b2": b2, "Wc1": Wc1,
            "bc1": bc1, "Wc2": Wc2, "bc2": bc2, "Wp1": Wp1,
            "bp1": bp1, "Wp2": Wp2, "bp2": bp2}
    concat_in = []
    for name in in_names:
        if name == "features":
            concat_in.append(np.ascontiguousarray(features, FEAT_NP))
        else:
            w = np.ascontiguousarray(vals[name], np.float32)
            concat_in.append(np.concatenate([w] * N_CORES, axis=0))
    outs = sharded(*concat_in)
    res = np.empty((n_full, T * 3), np.float32)
    shards = outs[0].addressable_shards

    def fetch(sh):
        res[sh.index[0]] = np.asarray(sh.data)

    list(_get_pool().map(fetch, shards))
    return res.reshape(n_full, T, 3)

